# revision 6
# baseline (speedup 1.0000x reference)
"""Sliding-window attention TRN2 kernel (nn_Attention_89764816486949).

Sharding: 8 cores = 4 head-groups x 2 batches. Core c handles batch (c % 2)
and heads [4*(c//2) .. 4*(c//2)+3]. Each core computes its partial output
projection outT [D, T] = sum_{its heads} w_out[h].T @ encoded[h].T; the host
sums the 4 partials per batch and transposes.

All matmuls run as float32r (TF32-like, ~1.4e-4 rel err, full PE rate).

Structure per head-pair pass: for each 512-token block tb, one k-loop
accumulates qT/kT/vT for both heads into 6 rotating PSUM banks; evictions
(RoPE for q/k, PE-transpose for v) then free the banks while the banded
attention for query block g = tb runs, so ScalarE softcap work overlaps the
next block's projection matmuls.
"""
import sys
sys.path.insert(0, '/opt/trn_rl_repo')

import numpy as np

B, T, D, N, H = 2, 2048, 2048, 16, 128
WINDOW = 1024
SOFT_CAP = 50.0
MAX_WAVELENGTH = 10000

HPC = 4            # heads per core
TB = 512           # token block (free dim of most matmuls)
NTB = T // TB      # 4
NK = D // 128      # 16 contraction tiles
NCORES = 8

_compiled = {}


def _build_nc():
    import concourse.bacc as bacc
    import concourse.mybir as mybir
    from concourse import tile

    F32 = mybir.dt.float32
    F32R = mybir.dt.float32r
    AF = mybir.ActivationFunctionType
    OP = mybir.AluOpType

    nc = bacc.Bacc(None, target_bir_lowering=False, debug=False)

    xt_d = nc.dram_tensor("xt", [D, T], F32R, kind="ExternalInput").ap()
    wq_d = nc.dram_tensor("wq", [HPC, D, H], F32R, kind="ExternalInput").ap()
    wk_d = nc.dram_tensor("wk", [HPC, D, H], F32R, kind="ExternalInput").ap()
    wv_d = nc.dram_tensor("wv", [HPC, D, H], F32R, kind="ExternalInput").ap()
    wo_d = nc.dram_tensor("wo", [HPC, H, D], F32R, kind="ExternalInput").ap()
    cos_d = nc.dram_tensor("ropecos", [H, T], F32, kind="ExternalInput").ap()
    sin_d = nc.dram_tensor("ropesin", [H, T], F32, kind="ExternalInput").ap()
    maskc_d = nc.dram_tensor("maskc", [128, 896], F32, kind="ExternalInput").ap()
    maskw_d = nc.dram_tensor("maskw", [128, 896], F32, kind="ExternalInput").ap()
    ones_d = nc.dram_tensor("ones", [128, 1], F32R, kind="ExternalInput").ap()
    onesrow_d = nc.dram_tensor("onesrow", [1, 128], F32R, kind="ExternalInput").ap()
    ident_d = nc.dram_tensor("ident", [128, 128], F32, kind="ExternalInput").ap()
    outt_d = nc.dram_tensor("outt", [D, T], F32, kind="ExternalOutput").ap()

    with tile.TileContext(nc) as tc:
        with tc.tile_pool(name="outer", bufs=1) as outer, \
             tc.tile_pool(name="psum", bufs=1, space="PSUM") as psp:
            cos_sb = outer.tile([H, T], F32, tag="cos")
            nc.sync.dma_start(out=cos_sb[:, :], in_=cos_d[:, :])
            sin_sb = outer.tile([H, T], F32, tag="sin")
            nc.sync.dma_start(out=sin_sb[:, :], in_=sin_d[:, :])
            maskc_sb = outer.tile([128, 896], F32, tag="maskc")
            nc.sync.dma_start(out=maskc_sb[:, :], in_=maskc_d[:, :])
            maskw_sb = outer.tile([128, 896], F32, tag="maskw")
            nc.sync.dma_start(out=maskw_sb[:, :], in_=maskw_d[:, :])
            ones_sb = outer.tile([128, 1], F32R, tag="ones")
            nc.sync.dma_start(out=ones_sb[:, :], in_=ones_d[:, :])
            onesrow_sb = outer.tile([1, 128], F32R, tag="onesrow")
            nc.sync.dma_start(out=onesrow_sb[:, :], in_=onesrow_d[:, :])
            ident_sb = outer.tile([128, 128], F32, tag="ident")
            nc.sync.dma_start(out=ident_sb[:, :], in_=ident_d[:, :])
            enc_sb = [outer.tile([H, T], F32R, tag=f"enc{h}", name=f"enc{h}")
                      for h in range(HPC)]

            def mmtile(name):
                return psp.tile([128, TB], F32, tag="mm", bufs=6, name=name)

            def attn_group(pp, hh, h, g, qT, kT, v_sb):
                t0 = g * TB
                jmin = max(0, t0 - (WINDOW - 1)) // 128
                jmax = (t0 + TB - 1) // 128
                pts = {}
                for j in range(jmin, jmax + 1):
                    stp = mmtile(f"st{h}_{g}_{j}")
                    nc.tensor.matmul(stp[:, :],
                                     kT[hh][:, j * 128:(j + 1) * 128],
                                     qT[hh][:, t0:t0 + TB],
                                     start=True, stop=True)
                    pt = pp.tile([128, TB], F32R, tag=f"pt{(j - jmin) % 4}",
                                 bufs=1, name=f"pt{h}_{g}_{j}")
                    nc.scalar.activation(pt[:, :], stp[:, :], AF.Tanh,
                                         scale=1.0 / SOFT_CAP)
                    nc.scalar.activation(pt[:, :], pt[:, :], AF.Exp,
                                         scale=SOFT_CAP)
                    r = j - 4 * g
                    m = 4 * g - j
                    if 0 <= r <= 3:        # causal diagonal tiles
                        off = 384 - 128 * r
                        nc.vector.tensor_tensor(
                            out=pt[:, :], in0=pt[:, :],
                            in1=maskc_sb[:, off:off + TB], op=OP.mult)
                    elif 5 <= m <= 8:      # sliding-window lower edge
                        off = 128 * m - 640
                        nc.vector.tensor_tensor(
                            out=pt[:, :], in0=pt[:, :],
                            in1=maskw_sb[:, off:off + TB], op=OP.mult)
                    pts[j] = pt
                encp = psp.tile([H, TB], F32, tag="encp", bufs=1,
                                name=f"encp{h}_{g}")
                for j in range(jmin, jmax + 1):
                    nc.tensor.matmul(encp[:, :],
                                     v_sb[hh][:, j * 128:(j + 1) * 128],
                                     pts[j][:, :],
                                     start=(j == jmin), stop=(j == jmax))
                sums = psp.tile([1, TB], F32, tag="sums", bufs=1,
                                name=f"sums{h}_{g}")
                for j in range(jmin, jmax + 1):
                    nc.tensor.matmul(sums[:, :], ones_sb[:, :], pts[j][:, :],
                                     start=(j == jmin), stop=(j == jmax))
                sums_sb = pp.tile([1, TB], F32R, tag="sums_sb", bufs=2,
                                  name=f"sums_sb{h}_{g}")
                nc.scalar.activation(sums_sb[:, :], sums[:, :], AF.Copy)
                rcb_ps = mmtile(f"rcb{h}_{g}")
                nc.tensor.matmul(rcb_ps[:, :], onesrow_sb[:, :],
                                 sums_sb[:, :], start=True, stop=True)
                recipb = pp.tile([128, TB], F32, tag="recipb", bufs=2,
                                 name=f"recipb{h}_{g}")
                nc.vector.reciprocal(recipb[:, :], rcb_ps[:, :])
                nc.vector.tensor_tensor(out=enc_sb[h][:, t0:t0 + TB],
                                        in0=encp[:, :], in1=recipb[:, :],
                                        op=OP.mult)

            for p in range(2):  # head pairs (2p, 2p+1)
                with tc.tile_pool(name=f"pass{p}", bufs=1) as pp:
                    wq_sb, wk_sb, wv_sb, qT, kT, v_sb = [], [], [], [], [], []
                    for hh in range(2):
                        h = 2 * p + hh
                        for wlist, wd, nm in ((wq_sb, wq_d, "wq"),
                                              (wk_sb, wk_d, "wk"),
                                              (wv_sb, wv_d, "wv")):
                            wt = pp.tile([128, NK * H], F32R, tag=f"{nm}{hh}",
                                         name=f"{nm}{hh}")
                            nc.sync.dma_start(
                                out=wt[:, :].rearrange("p (k j) -> p k j", j=H),
                                in_=wd[h].rearrange("(k p) j -> p k j", p=128))
                            wlist.append(wt)
                        qT.append(pp.tile([H, T], F32R, tag=f"qT{hh}",
                                          name=f"qT{hh}"))
                        kT.append(pp.tile([H, T], F32R, tag=f"kT{hh}",
                                          name=f"kT{hh}"))
                        v_sb.append(pp.tile([128, T], F32R, tag=f"v{hh}",
                                            name=f"v{hh}"))

                    for tb in range(NTB):
                        # ---- projection k-loop: 6 rotating accumulators ----
                        pq = [mmtile(f"pq{hh}_{tb}") for hh in range(2)]
                        pk = [mmtile(f"pk{hh}_{tb}") for hh in range(2)]
                        pv = [mmtile(f"pv{hh}_{tb}") for hh in range(2)]
                        for k in range(NK):
                            xt = pp.tile([128, TB], F32R, tag="xt", bufs=6,
                                         name=f"xt{tb}_{k}")
                            nc.sync.dma_start(
                                out=xt[:, :],
                                in_=xt_d[k * 128:(k + 1) * 128,
                                         tb * TB:(tb + 1) * TB])
                            st = (k == 0)
                            sp = (k == NK - 1)
                            for hh in range(2):
                                nc.tensor.matmul(
                                    pq[hh][:, :],
                                    wq_sb[hh][:, k * H:(k + 1) * H],
                                    xt[:, :], start=st, stop=sp)
                                nc.tensor.matmul(
                                    pk[hh][:, :],
                                    wk_sb[hh][:, k * H:(k + 1) * H],
                                    xt[:, :], start=st, stop=sp)
                                nc.tensor.matmul(
                                    pv[hh][:, :],
                                    wv_sb[hh][:, k * H:(k + 1) * H],
                                    xt[:, :], start=st, stop=sp)
                        # ---- evictions ----
                        cosb = cos_sb[:, tb * TB:(tb + 1) * TB]
                        sinb = sin_sb[:, tb * TB:(tb + 1) * TB]
                        for hh in range(2):
                            # RoPE for q and k
                            for ps, dst in ((pq[hh], qT[hh]), (pk[hh], kT[hh])):
                                dslice = dst[:, tb * TB:(tb + 1) * TB]
                                raw = pp.tile([128, TB], F32, tag="raw",
                                              bufs=2, name="raw")
                                nc.scalar.activation(raw[:, :], ps[:, :],
                                                     AF.Copy)
                                rot = pp.tile([128, TB], F32, tag="rot",
                                              bufs=2, name="rot")
                                nc.gpsimd.dma_start(out=rot[0:64, :],
                                                    in_=raw[64:128, :])
                                nc.gpsimd.dma_start(out=rot[64:128, :],
                                                    in_=raw[0:64, :])
                                t1 = pp.tile([128, TB], F32, tag="t1",
                                             bufs=2, name="t1")
                                nc.vector.tensor_tensor(
                                    out=t1[:, :], in0=rot[:, :], in1=sinb,
                                    op=OP.mult)
                                nc.vector.tensor_tensor(
                                    out=dslice, in0=ps[:, :], in1=cosb,
                                    op=OP.mult)
                                nc.vector.tensor_tensor(
                                    out=dslice, in0=dslice.bitcast(F32),
                                    in1=t1[:, :], op=OP.add)
                            # v via PE transpose of vT
                            vtmp = pp.tile([128, TB], F32, tag="vtmp",
                                           bufs=2, name="vtmp")
                            nc.vector.tensor_copy(vtmp[:, :], pv[hh][:, :])
                            for i in range(4):
                                tp = psp.tile([128, 128], F32, tag="mm",
                                              bufs=6, name=f"tp{hh}_{tb}_{i}")
                                nc.tensor.transpose(
                                    tp[:, :], vtmp[:, i * 128:(i + 1) * 128],
                                    ident_sb[:, :])
                                nc.vector.tensor_copy(
                                    v_sb[hh][:, (tb * 4 + i) * 128:
                                             (tb * 4 + i + 1) * 128],
                                    tp[:, :])
                        # ---- attention for query block g = tb ----
                        for hh in range(2):
                            attn_group(pp, hh, 2 * p + hh, tb, qT, kT, v_sb)

            # ---- output projection ----
            with tc.tile_pool(name="oproj", bufs=1) as op_pool:
                wo_sb = []
                for h in range(HPC):
                    wt = op_pool.tile([H, D], F32R, tag=f"wo{h}", name=f"wo{h}")
                    nc.sync.dma_start(out=wt[:, :], in_=wo_d[h])
                    wo_sb.append(wt)
                for d in range(D // 128):
                    for tb in range(NTB):
                        po = mmtile(f"po{d}_{tb}")
                        for h in range(HPC):
                            nc.tensor.matmul(
                                po[:, :],
                                wo_sb[h][:, d * 128:(d + 1) * 128],
                                enc_sb[h][:, tb * TB:(tb + 1) * TB],
                                start=(h == 0), stop=(h == HPC - 1))
                        osb = op_pool.tile([128, TB], F32, tag="osb", bufs=4,
                                           name=f"osb{d}_{tb}")
                        nc.scalar.activation(osb[:, :], po[:, :], AF.Copy)
                        nc.sync.dma_start(
                            out=outt_d[d * 128:(d + 1) * 128,
                                       tb * TB:(tb + 1) * TB],
                            in_=osb[:, :])

    nc.compile()
    return nc


def _host_inputs(x, w_qkv, w_out, segment_pos):
    """Build the 8 per-core input maps."""
    scale = np.float32(H ** -0.5)
    in_maps = []
    # rope tables per batch (mirror the reference's fp32 arithmetic)
    fraction = (2.0 * np.arange(H // 2, dtype=np.float32) /
                np.float32(H)).astype(np.float32)
    timescale = np.power(np.float32(MAX_WAVELENGTH), fraction).astype(np.float32)
    tabs = []
    for b in range(B):
        ang = (segment_pos[b][:, None].astype(np.float32) / timescale[None, :])
        ang = ang.astype(np.float32)          # [T, 64]
        c = np.cos(ang).astype(np.float32).T  # [64, T]
        s = np.sin(ang).astype(np.float32).T
        cos_full = np.ascontiguousarray(np.concatenate([c, c], axis=0))
        sgn_sin = np.ascontiguousarray(np.concatenate([-s, s], axis=0))
        tabs.append((cos_full, sgn_sin))

    ds = np.arange(128)[:, None]
    u = np.arange(896)[None, :]
    maskc = (u - 384 >= ds).astype(np.float32)
    maskw = (u <= ds + 383).astype(np.float32)
    ones = np.ones((128, 1), np.float32)
    onesrow = np.ones((1, 128), np.float32)
    ident = np.eye(128, dtype=np.float32)

    xts = [np.ascontiguousarray(x[b].T) for b in range(B)]

    for c in range(NCORES):
        b = c % 2
        hg = c // 2
        hs = hg * HPC
        wq = np.ascontiguousarray(w_qkv[0, hs:hs + HPC] * scale)
        wk = np.ascontiguousarray(w_qkv[1, hs:hs + HPC])
        wv = np.ascontiguousarray(w_qkv[2, hs:hs + HPC])
        wo = np.ascontiguousarray(w_out[hs:hs + HPC])
        in_maps.append({
            "xt": xts[b], "wq": wq, "wk": wk, "wv": wv, "wo": wo,
            "ropecos": tabs[b][0], "ropesin": tabs[b][1],
            "maskc": maskc, "maskw": maskw, "ones": ones, "onesrow": onesrow,
            "ident": ident,
        })
    return in_maps


def kernel(x, w_qkv, w_out, segment_pos, attn_mask, _trace=False):
    from concourse.bass_utils import run_bass_kernel_spmd

    x = np.asarray(x, dtype=np.float32)
    w_qkv = np.asarray(w_qkv, dtype=np.float32)
    w_out = np.asarray(w_out, dtype=np.float32)
    segment_pos = np.asarray(segment_pos)

    if "nc" not in _compiled:
        _compiled["nc"] = _build_nc()
    nc = _compiled["nc"]

    in_maps = _host_inputs(x, w_qkv, w_out, segment_pos)
    r = run_bass_kernel_spmd(nc, in_maps, core_ids=list(range(NCORES)),
                             trace=_trace)
    _compiled["last_results"] = r

    out = np.zeros((B, T, D), np.float32)
    for b in range(B):
        acc = np.zeros((D, T), np.float64)
        for c in range(b, NCORES, 2):
            acc += r.results[c]["outt"]
        out[b] = acc.T.astype(np.float32)
    return out
